# revision 15
# baseline (speedup 1.0000x reference)
"""BinaryLinear forward on 8 Trainium2 NeuronCores.

out = x @ (sign(W) * scale).T + bias
  x:      [4, 2048, 4096] f32
  W:      [16384, 4096]   f32
  scale:  [1]             f32
  bias:   [16384]         f32
  out:    [4, 2048, 16384] f32

Strategy (column-parallel / tensor-parallel over out_features):
  - sign(W) is exactly representable in bf16 (+-1), so the matmul runs at
    the bf16 PE peak (78.6 TF/s/core) with no weight quantization error.
  - scale is folded into x on the host: (x*scale) @ sign(W).T == x @ (sign(W)*scale).T
  - x is cast to bf16 (adds ~1e-3 relative error vs the f32 reference).
  - Each core computes out[:, c*2048:(c+1)*2048] = xT.T @ wT_shard + bias_shard.
  - Per core: M=8192 tokens, K=4096, N=2048. The W^T shard is SBUF-resident
    in fp8e4 (+-1 exact; streamed as the matmul moving operand at full rate,
    which also keeps the FWL weight loads of the bf16 x stationary operand
    fully hidden). x streams in [128, 4096] token tiles, host-packed for
    1 KiB-contiguous DMA chunks; PSUM accumulates over 32 K-tiles; bias-add
    is fused into the PSUM->SBUF eviction on the vector engine.
  Measured: ~1.794 ms HW exec (97.5% of the 78.6 TF/s bf16 PE peak),
  rel absmax err 1.7e-3.
"""

import sys

if "/opt/trn_rl_repo" not in sys.path:
    sys.path.insert(0, "/opt/trn_rl_repo")

import numpy as np
import ml_dtypes

N_CORES = 8
B, S, K = 4, 2048, 4096
OUT_F = 16384
M = B * S                 # 8192 tokens
NS = OUT_F // N_CORES     # 2048 out-features per core
P = 128
FD = 512                  # matmul free dim (one PSUM bank)

_compiled = None


def _kig(kt):
    return 4 if kt % 4 == 0 else (2 if kt % 2 == 0 else 1)


def build_program(m=M, k=K, ns=NS):
    import concourse.mybir as mybir
    import concourse.tile as tile
    from concourse import bacc

    kt = k // P    # K-tiles
    mt = m // P    # token tiles
    nb = ns // FD  # PSUM banks per token tile
    kig = _kig(kt)  # K-tiles packed contiguously per partition chunk
    ko = kt // kig

    nc = bacc.Bacc("TRN2", target_bir_lowering=False, debug=False)

    # x is host-packed as xB[p, g, mi, ki*128+j] = x^T[(g*kig+ki)*128+p, mi*128+j]
    # so each x-tile DMA reads kig*128*2-byte contiguous chunks per partition.
    xB = nc.dram_tensor(
        "xB", [P, ko, mt, kig * P], mybir.dt.bfloat16, kind="ExternalInput"
    )
    wT = nc.dram_tensor("wT", [k, ns], mybir.dt.float8e4, kind="ExternalInput")
    bias_bc = nc.dram_tensor("bias_bc", [P, ns], mybir.dt.float32, kind="ExternalInput")
    out = nc.dram_tensor("out", [m, ns], mybir.dt.float32, kind="ExternalOutput")

    w_ap = wT.rearrange("(kt p) n -> p kt n", p=P)  # [128, kt, ns]

    with tile.TileContext(nc) as tc:
        with (
            tc.tile_pool(name="const", bufs=1) as const,
            tc.tile_pool(name="xin", bufs=4) as xin,
            tc.tile_pool(name="acc", bufs=3) as accp,
            tc.tile_pool(name="psum", bufs=2, space="PSUM") as psump,
        ):
            # Weights live SBUF-resident in fp8 (+-1 is exact); the matmul
            # streams them as the moving operand at full rate. Load is split
            # per K-tile, first K-tile ahead of everything, so the PE can
            # start ~4us in instead of waiting for the whole 8.4 MB.
            w_sb = const.tile([P, kt, ns], mybir.dt.float8e4, name="w_sb")
            bias_sb = const.tile([P, ns], mybir.dt.float32, name="bias_sb")
            xm0 = xin.tile([P, ko, kig * P], mybir.dt.bfloat16, name="xm")
            xm1 = None

            # PE pre-warm: a few dummy matmuls on memset tiles run during the
            # initial DMA wait, tripping the HAM clock gate to 2.4 GHz before
            # the real stream starts (saves ~8 cold matmuls at 1.2 GHz).
            dW = const.tile([P, P], mybir.dt.bfloat16, name="dW")
            dM = const.tile([P, FD], mybir.dt.float8e4, name="dM")
            nc.vector.memset(dW[:], 0.0)
            nc.vector.memset(dM[:], 0.0)
            warm = psump.tile([P, FD], mybir.dt.float32, name="ps0")
            for _ in range(8):
                nc.tensor.matmul(warm[:], lhsT=dW[:], rhs=dM[:], start=True, stop=True)
            # Startup DMA choreography. The PE consumes one K-tile of weights
            # every ~864ns while the single DMA stream delivers one every
            # ~660ns, so the weight lead grows ~0.2us per K-tile — but every
            # non-weight insertion (x chunks, bias) spends that lead. Place
            # the x-tile-0 chunks, bias quarters, and the x-tile-1 prefetch
            # just-in-time, each well before its consumer but behind enough
            # weight chunks that the PE never stalls.
            nxc = min(4, ko)
            xc = ko // nxc
            nc.sync.dma_start(out=xm0[:, 0:xc, :], in_=xB[:, 0:xc, 0, :])
            nc.sync.dma_start(out=w_sb[:, 0:1, :], in_=w_ap[:, 0:1, :])
            if kt >= 32 and nxc == 4 and mt > 1:
                ins = {}  # after-weight-chunk g -> list of DMA thunks
                for ci in range(1, nxc):
                    ins.setdefault(6 * ci - 1, []).append(
                        lambda ci=ci: nc.sync.dma_start(
                            out=xm0[:, ci * xc : (ci + 1) * xc, :],
                            in_=xB[:, ci * xc : (ci + 1) * xc, 0, :],
                        )
                    )
                bias_q = ns // 4
                for i in range(4):
                    ins.setdefault(20 + i, []).append(
                        lambda i=i: nc.sync.dma_start(
                            out=bias_sb[:, i * bias_q : (i + 1) * bias_q],
                            in_=bias_bc[:, i * bias_q : (i + 1) * bias_q],
                        )
                    )
                xm1 = xin.tile([P, ko, kig * P], mybir.dt.bfloat16, name="xm")
                ins.setdefault(kt - 5, []).append(
                    lambda: nc.sync.dma_start(
                        out=xm1[:, : ko // 2, :], in_=xB[:, : ko // 2, 1, :]
                    )
                )
                for g in range(1, kt):
                    nc.sync.dma_start(
                        out=w_sb[:, g : g + 1, :], in_=w_ap[:, g : g + 1, :]
                    )
                    for thunk in ins.get(g, ()):
                        thunk()
                nc.sync.dma_start(
                    out=xm1[:, ko // 2 :, :], in_=xB[:, ko // 2 :, 1, :]
                )
            else:
                for ci in range(1, nxc):
                    nc.sync.dma_start(
                        out=xm0[:, ci * xc : (ci + 1) * xc, :],
                        in_=xB[:, ci * xc : (ci + 1) * xc, 0, :],
                    )
                for g in range(1, kt):
                    nc.sync.dma_start(
                        out=w_sb[:, g : g + 1, :], in_=w_ap[:, g : g + 1, :]
                    )
                nc.sync.dma_start(out=bias_sb[:], in_=bias_bc[:])
                if mt > 1:
                    xm1 = xin.tile([P, ko, kig * P], mybir.dt.bfloat16, name="xm")
                    nc.sync.dma_start(out=xm1[:], in_=xB[:, :, 1, :])

            for mi in range(mt):
                if mi == 0:
                    xm = xm0
                elif mi == 1:
                    xm = xm1
                else:
                    xm = xin.tile([P, ko, kig * P], mybir.dt.bfloat16, name="xm")
                    nc.sync.dma_start(out=xm[:], in_=xB[:, :, mi, :])

                psums = [
                    psump.tile([P, FD], mybir.dt.float32, name=f"ps{j}")
                    for j in range(nb)
                ]
                for ki in range(kt):
                    for j in range(nb):
                        nc.tensor.matmul(
                            psums[j][:],
                            lhsT=xm[:, ki // kig, (ki % kig) * P : (ki % kig + 1) * P],
                            rhs=w_sb[:, ki, j * FD : (j + 1) * FD],
                            start=(ki == 0),
                            stop=(ki == kt - 1),
                        )

                ot = accp.tile([P, ns], mybir.dt.float32, name="ot")
                for j in range(nb):
                    nc.vector.tensor_tensor(
                        out=ot[:, j * FD : (j + 1) * FD],
                        in0=psums[j][:],
                        in1=bias_sb[:, j * FD : (j + 1) * FD],
                        op=mybir.AluOpType.add,
                    )
                    nc.sync.dma_start(
                        out=out[mi * P : (mi + 1) * P, j * FD : (j + 1) * FD],
                        in_=ot[:, j * FD : (j + 1) * FD],
                    )

    nc.compile()
    return nc


def pack_x(xT_bf16, m=M, k=K):
    """xT [k, m] bf16 -> xB [128, k/(128*kig), m/128, kig*128] with
    kig*256-byte contiguous chunks per (partition, group) for fast DMA."""
    kt = k // P
    mt = m // P
    kig = _kig(kt)
    ko = kt // kig
    # xB[p, g, mi, ki*128+j] = xT[(g*kig+ki)*128+p, mi*128+j]
    v = xT_bf16.reshape(ko, kig, P, mt, P)
    return np.ascontiguousarray(v.transpose(2, 0, 3, 1, 4).reshape(P, ko, mt, kig * P))


def prepare_in_maps(x, weight, scale, bias):
    bf16 = ml_dtypes.bfloat16
    s = float(np.asarray(scale).reshape(-1)[0])
    xs = np.asarray(x, dtype=np.float32).reshape(M, K)
    if s != 1.0:
        xs = xs * s
    xT = np.ascontiguousarray(xs.T).astype(bf16)  # [K, M]
    xBp = pack_x(xT)

    w = np.asarray(weight, dtype=np.float32)
    wbin = np.where(w >= 0, np.float32(1), np.float32(-1))  # sign, 0 -> +1
    b = np.asarray(bias, dtype=np.float32)

    in_maps = []
    for c in range(N_CORES):
        wsh = wbin[c * NS : (c + 1) * NS, :]                  # [NS, K]
        wTsh = np.ascontiguousarray(wsh.T).astype(ml_dtypes.float8_e4m3)  # [K, NS]
        bsh = b[c * NS : (c + 1) * NS]
        bias_bc = np.ascontiguousarray(
            np.broadcast_to(bsh[None, :], (P, NS)), dtype=np.float32
        )
        in_maps.append({"xB": xBp, "wT": wTsh, "bias_bc": bias_bc})
    return in_maps


def gather(results):
    shards = [np.asarray(results[c]["out"]) for c in range(N_CORES)]  # [M, NS] each
    return np.concatenate(shards, axis=1).reshape(B, S, OUT_F)


def run(in_maps, trace=False, retries=2, **kwargs):
    global _compiled
    import time as _time

    from concourse import bass_utils

    if _compiled is None:
        _compiled = build_program()
    last_err = None
    for attempt in range(retries + 1):
        try:
            return bass_utils.run_bass_kernel_spmd(
                _compiled, in_maps, core_ids=list(range(N_CORES)), trace=trace, **kwargs
            )
        except Exception as e:  # transient NRT device wedge: retry
            last_err = e
            if attempt < retries:
                _time.sleep(5)
    raise last_err


def kernel(x, weight, scale, bias):
    res = run(prepare_in_maps(x, weight, scale, bias))
    return gather(res.results)


# revision 16
# speedup vs baseline: 1.0017x; 1.0017x over previous
"""BinaryLinear forward on 8 Trainium2 NeuronCores.

out = x @ (sign(W) * scale).T + bias
  x:      [4, 2048, 4096] f32
  W:      [16384, 4096]   f32
  scale:  [1]             f32
  bias:   [16384]         f32
  out:    [4, 2048, 16384] f32

Strategy (column-parallel / tensor-parallel over out_features):
  - sign(W) is exactly representable in bf16 (+-1), so the matmul runs at
    the bf16 PE peak (78.6 TF/s/core) with no weight quantization error.
  - scale is folded into x on the host: (x*scale) @ sign(W).T == x @ (sign(W)*scale).T
  - x is cast to bf16 (adds ~1e-3 relative error vs the f32 reference).
  - Each core computes out[:, c*2048:(c+1)*2048] = xT.T @ wT_shard + bias_shard.
  - Per core: M=8192 tokens, K=4096, N=2048. The W^T shard is SBUF-resident
    in fp8e4 (+-1 exact; streamed as the matmul moving operand at full rate,
    which also keeps the FWL weight loads of the bf16 x stationary operand
    fully hidden). x streams in [128, 4096] token tiles, host-packed for
    1 KiB-contiguous DMA chunks; PSUM accumulates over 32 K-tiles; bias-add
    is fused into the PSUM->SBUF eviction on the vector engine.
  Measured: ~1.794 ms HW exec (97.5% of the 78.6 TF/s bf16 PE peak),
  rel absmax err 1.7e-3.
"""

import sys

if "/opt/trn_rl_repo" not in sys.path:
    sys.path.insert(0, "/opt/trn_rl_repo")

import numpy as np
import ml_dtypes

N_CORES = 8
B, S, K = 4, 2048, 4096
OUT_F = 16384
M = B * S                 # 8192 tokens
NS = OUT_F // N_CORES     # 2048 out-features per core
P = 128
FD = 512                  # matmul free dim (one PSUM bank)

_compiled = None


def _kig(kt):
    return 4 if kt % 4 == 0 else (2 if kt % 2 == 0 else 1)


def build_program(m=M, k=K, ns=NS):
    import concourse.mybir as mybir
    import concourse.tile as tile
    from concourse import bacc

    kt = k // P    # K-tiles
    mt = m // P    # token tiles
    nb = ns // FD  # PSUM banks per token tile
    kig = _kig(kt)  # K-tiles packed contiguously per partition chunk
    ko = kt // kig

    nc = bacc.Bacc("TRN2", target_bir_lowering=False, debug=False)

    # x is host-packed as xB[p, g, mi, ki*128+j] = x^T[(g*kig+ki)*128+p, mi*128+j]
    # so each x-tile DMA reads kig*128*2-byte contiguous chunks per partition.
    xB = nc.dram_tensor(
        "xB", [P, ko, mt, kig * P], mybir.dt.bfloat16, kind="ExternalInput"
    )
    wT = nc.dram_tensor("wT", [k, ns], mybir.dt.float8e4, kind="ExternalInput")
    bias_bc = nc.dram_tensor("bias_bc", [P, ns], mybir.dt.float32, kind="ExternalInput")
    out = nc.dram_tensor("out", [m, ns], mybir.dt.float32, kind="ExternalOutput")

    w_ap = wT.rearrange("(kt p) n -> p kt n", p=P)  # [128, kt, ns]

    with tile.TileContext(nc) as tc:
        with (
            tc.tile_pool(name="const", bufs=1) as const,
            tc.tile_pool(name="xin", bufs=4) as xin,
            tc.tile_pool(name="acc", bufs=3) as accp,
            tc.tile_pool(name="psum", bufs=2, space="PSUM") as psump,
        ):
            # Weights live SBUF-resident in fp8 (+-1 is exact); the matmul
            # streams them as the moving operand at full rate. Load is split
            # per K-tile, first K-tile ahead of everything, so the PE can
            # start ~4us in instead of waiting for the whole 8.4 MB.
            w_sb = const.tile([P, kt, ns], mybir.dt.float8e4, name="w_sb")
            bias_sb = const.tile([P, ns], mybir.dt.float32, name="bias_sb")
            xm0 = xin.tile([P, ko, kig * P], mybir.dt.bfloat16, name="xm")
            xm1 = None

            # PE pre-warm: a few dummy matmuls on memset tiles run during the
            # initial DMA wait, tripping the HAM clock gate to 2.4 GHz before
            # the real stream starts (saves ~8 cold matmuls at 1.2 GHz).
            dW = const.tile([P, P], mybir.dt.bfloat16, name="dW")
            dM = const.tile([P, FD], mybir.dt.float8e4, name="dM")
            nc.vector.memset(dW[:], 0.0)
            nc.vector.memset(dM[:], 0.0)
            warm = psump.tile([P, FD], mybir.dt.float32, name="ps0")
            for _ in range(8):
                nc.tensor.matmul(warm[:], lhsT=dW[:], rhs=dM[:], start=True, stop=True)
            # Startup DMA choreography. The PE consumes one K-tile of weights
            # every ~864ns while the single DMA stream delivers one every
            # ~660ns, so the weight lead grows ~0.2us per K-tile — but every
            # non-weight insertion (x chunks, bias) spends that lead. Place
            # the x-tile-0 chunks, bias quarters, and the x-tile-1 prefetch
            # just-in-time, each well before its consumer but behind enough
            # weight chunks that the PE never stalls.
            nxc = min(4, ko)
            xc = ko // nxc
            nc.sync.dma_start(out=xm0[:, 0:xc, :], in_=xB[:, 0:xc, 0, :])
            nc.sync.dma_start(out=w_sb[:, 0:1, :], in_=w_ap[:, 0:1, :])
            if kt >= 32 and nxc == 4 and mt > 1:
                ins = {}  # after-weight-chunk g -> list of DMA thunks
                for ci in range(1, nxc):
                    ins.setdefault(6 * ci + 3, []).append(
                        lambda ci=ci: nc.sync.dma_start(
                            out=xm0[:, ci * xc : (ci + 1) * xc, :],
                            in_=xB[:, ci * xc : (ci + 1) * xc, 0, :],
                        )
                    )
                bias_q = ns // 4
                for i in range(4):
                    ins.setdefault(23 + i, []).append(
                        lambda i=i: nc.sync.dma_start(
                            out=bias_sb[:, i * bias_q : (i + 1) * bias_q],
                            in_=bias_bc[:, i * bias_q : (i + 1) * bias_q],
                        )
                    )
                xm1 = xin.tile([P, ko, kig * P], mybir.dt.bfloat16, name="xm")
                ins.setdefault(kt - 5, []).append(
                    lambda: nc.sync.dma_start(
                        out=xm1[:, : ko // 2, :], in_=xB[:, : ko // 2, 1, :]
                    )
                )
                for g in range(1, kt):
                    nc.sync.dma_start(
                        out=w_sb[:, g : g + 1, :], in_=w_ap[:, g : g + 1, :]
                    )
                    for thunk in ins.get(g, ()):
                        thunk()
                nc.sync.dma_start(
                    out=xm1[:, ko // 2 :, :], in_=xB[:, ko // 2 :, 1, :]
                )
            else:
                for ci in range(1, nxc):
                    nc.sync.dma_start(
                        out=xm0[:, ci * xc : (ci + 1) * xc, :],
                        in_=xB[:, ci * xc : (ci + 1) * xc, 0, :],
                    )
                for g in range(1, kt):
                    nc.sync.dma_start(
                        out=w_sb[:, g : g + 1, :], in_=w_ap[:, g : g + 1, :]
                    )
                nc.sync.dma_start(out=bias_sb[:], in_=bias_bc[:])
                if mt > 1:
                    xm1 = xin.tile([P, ko, kig * P], mybir.dt.bfloat16, name="xm")
                    nc.sync.dma_start(out=xm1[:], in_=xB[:, :, 1, :])

            for mi in range(mt):
                if mi == 0:
                    xm = xm0
                elif mi == 1:
                    xm = xm1
                else:
                    xm = xin.tile([P, ko, kig * P], mybir.dt.bfloat16, name="xm")
                    nc.sync.dma_start(out=xm[:], in_=xB[:, :, mi, :])

                psums = [
                    psump.tile([P, FD], mybir.dt.float32, name=f"ps{j}")
                    for j in range(nb)
                ]
                for ki in range(kt):
                    for j in range(nb):
                        nc.tensor.matmul(
                            psums[j][:],
                            lhsT=xm[:, ki // kig, (ki % kig) * P : (ki % kig + 1) * P],
                            rhs=w_sb[:, ki, j * FD : (j + 1) * FD],
                            start=(ki == 0),
                            stop=(ki == kt - 1),
                        )

                ot = accp.tile([P, ns], mybir.dt.float32, name="ot")
                for j in range(nb):
                    nc.vector.tensor_tensor(
                        out=ot[:, j * FD : (j + 1) * FD],
                        in0=psums[j][:],
                        in1=bias_sb[:, j * FD : (j + 1) * FD],
                        op=mybir.AluOpType.add,
                    )
                    nc.sync.dma_start(
                        out=out[mi * P : (mi + 1) * P, j * FD : (j + 1) * FD],
                        in_=ot[:, j * FD : (j + 1) * FD],
                    )

    nc.compile()
    return nc


def pack_x(xT_bf16, m=M, k=K):
    """xT [k, m] bf16 -> xB [128, k/(128*kig), m/128, kig*128] with
    kig*256-byte contiguous chunks per (partition, group) for fast DMA."""
    kt = k // P
    mt = m // P
    kig = _kig(kt)
    ko = kt // kig
    # xB[p, g, mi, ki*128+j] = xT[(g*kig+ki)*128+p, mi*128+j]
    v = xT_bf16.reshape(ko, kig, P, mt, P)
    return np.ascontiguousarray(v.transpose(2, 0, 3, 1, 4).reshape(P, ko, mt, kig * P))


def prepare_in_maps(x, weight, scale, bias):
    bf16 = ml_dtypes.bfloat16
    s = float(np.asarray(scale).reshape(-1)[0])
    xs = np.asarray(x, dtype=np.float32).reshape(M, K)
    if s != 1.0:
        xs = xs * s
    xT = np.ascontiguousarray(xs.T).astype(bf16)  # [K, M]
    xBp = pack_x(xT)

    w = np.asarray(weight, dtype=np.float32)
    wbin = np.where(w >= 0, np.float32(1), np.float32(-1))  # sign, 0 -> +1
    b = np.asarray(bias, dtype=np.float32)

    in_maps = []
    for c in range(N_CORES):
        wsh = wbin[c * NS : (c + 1) * NS, :]                  # [NS, K]
        wTsh = np.ascontiguousarray(wsh.T).astype(ml_dtypes.float8_e4m3)  # [K, NS]
        bsh = b[c * NS : (c + 1) * NS]
        bias_bc = np.ascontiguousarray(
            np.broadcast_to(bsh[None, :], (P, NS)), dtype=np.float32
        )
        in_maps.append({"xB": xBp, "wT": wTsh, "bias_bc": bias_bc})
    return in_maps


def gather(results):
    shards = [np.asarray(results[c]["out"]) for c in range(N_CORES)]  # [M, NS] each
    return np.concatenate(shards, axis=1).reshape(B, S, OUT_F)


def run(in_maps, trace=False, retries=2, **kwargs):
    global _compiled
    import time as _time

    from concourse import bass_utils

    if _compiled is None:
        _compiled = build_program()
    last_err = None
    for attempt in range(retries + 1):
        try:
            return bass_utils.run_bass_kernel_spmd(
                _compiled, in_maps, core_ids=list(range(N_CORES)), trace=trace, **kwargs
            )
        except Exception as e:  # transient NRT device wedge: retry
            last_err = e
            if attempt < retries:
                _time.sleep(5)
    raise last_err


def kernel(x, weight, scale, bias):
    res = run(prepare_in_maps(x, weight, scale, bias))
    return gather(res.results)
